# revision 2
# baseline (speedup 1.0000x reference)
"""Trainium2 Bass kernel for nn_AdditiveIntervention.

Reference computation (B=512, N=1024, D=FUSE=1024, A=256):
    q = fuse_rep @ Wq                               # [B, A]
    k = confounder_set @ Wk                         # [N, A]
    scores[b,n] = sum_a wt[a] * tanh(q[b,a]+k[n,a]) # [B, N]
    attn = softmax(scores, axis=1)
    out = (attn * probs) @ confounder_set           # [B, D]

Sharding: data-parallel over B across 8 NeuronCores (64 rows each); the
confounder set, probabilities and weights are replicated.

Per-core device algorithm (a on partitions, 2 half-tiles of 128):
    qT[a, b]  = Wq[:, a].T @ fuse_rep_local.T      (PE, PSUM accum over FUSE)
    kT[a, n]  = Wk[:, a].T @ confT                 (PE)
    for each group of G batch rows:
        DVE: add[a, n] = kT[a, n] + qT[a, b]       (tensor_scalar, per-part scalar)
        ACT: tanh over the whole [128, G*1024] buffer (batched, one instr)
        PE : scores[b, n-chunk] += onehot_b(wt).T @ tanh_tile
             (lhsT one-hot column b carrying wt -> accumulates row b in PSUM)
    softmax along free dim on [64, 1024] scores (DVE max, ACT exp+accum sum)
    w = exp * probs_bc; wT via PE transpose; out = wT.T @ conf (PE);
    final scale by 1/sum fused into the PSUM->SBUF copy (ACT scale).
"""

import numpy as np

from concourse import bacc, bass, tile
import concourse.mybir as mybir
from concourse.bass_utils import run_bass_kernel_spmd
from concourse.masks import make_identity

F32 = mybir.dt.float32
AF = mybir.ActivationFunctionType

B, N, D, FUSE, A = 512, 1024, 1024, 1024, 256
M = 8            # cores
BL = B // M      # 64 local batch rows per core
NH = A // 128    # 2 a-half tiles
G = 4            # batch rows per tanh batch
NCHUNK = 512     # fp32 matmul moving-operand max


def build_kernel(g: int = G):
    nc = bacc.Bacc("TRN2", target_bir_lowering=False, debug=False)

    conf = nc.dram_tensor("conf", [N, D], F32, kind="ExternalInput")
    confT = nc.dram_tensor("confT", [D, N], F32, kind="ExternalInput")
    frT = nc.dram_tensor("frT", [FUSE, BL], F32, kind="ExternalInput")
    Wq = nc.dram_tensor("Wq", [FUSE, A], F32, kind="ExternalInput")
    Wk = nc.dram_tensor("Wk", [D, A], F32, kind="ExternalInput")
    probs = nc.dram_tensor("probs", [1, N], F32, kind="ExternalInput")
    wtT = nc.dram_tensor("wtT", [128, NH], F32, kind="ExternalInput")
    out = nc.dram_tensor("out", [BL, D], F32, kind="ExternalOutput")

    kt_f = FUSE // 128   # contraction tiles
    nt = N // 128        # n tiles (for final matmul / transpose)
    ngroups = BL // g

    with tile.TileContext(nc) as tc:
        with (
            tc.tile_pool(name="persist", bufs=1) as pp,
            tc.tile_pool(name="scoreps", bufs=1, space="PSUM") as scorepool,
        ):
            conf_sb = pp.tile([128, nt, D], F32)
            kT_sb = pp.tile([128, NH, N], F32)
            qT_sb = pp.tile([128, NH, BL], F32)
            onehot = pp.tile([128, NH, BL, BL], F32)
            probs_bc = pp.tile([BL, N], F32)
            wtT_sb = pp.tile([128, NH], F32)

            scores_ps = [
                scorepool.tile([BL, NCHUNK], F32, tag=f"sc{c}", name=f"scores_ps{c}")
                for c in range(N // NCHUNK)
            ]

            # ---------------- setup ----------------
            with (
                tc.tile_pool(name="setup", bufs=1) as sp,
                tc.tile_pool(name="setps", bufs=2, space="PSUM") as setps,
            ):
                confT_sb = sp.tile([128, kt_f, N], F32)
                Wq_sb = sp.tile([128, kt_f, A], F32)
                Wk_sb = sp.tile([128, kt_f, A], F32)
                frT_sb = sp.tile([128, kt_f, BL], F32)
                probs_sb = sp.tile([1, N], F32)
                ones_sb = sp.tile([1, BL], F32)

                nc.sync.dma_start(
                    confT_sb[:], confT[:].rearrange("(t p) n -> p t n", p=128)
                )
                nc.sync.dma_start(
                    Wq_sb[:], Wq[:].rearrange("(t p) a -> p t a", p=128)
                )
                nc.sync.dma_start(
                    Wk_sb[:], Wk[:].rearrange("(t p) a -> p t a", p=128)
                )
                nc.sync.dma_start(
                    frT_sb[:], frT[:].rearrange("(t p) b -> p t b", p=128)
                )
                nc.sync.dma_start(probs_sb[:], probs[:])
                nc.sync.dma_start(wtT_sb[:], wtT[:])
                nc.sync.dma_start(
                    conf_sb[:], conf[:].rearrange("(t p) d -> p t d", p=128)
                )
                nc.gpsimd.memset(ones_sb[:], 1.0)

                # qT[a_half] = Wq[:, half].T @ frT  -> [128, BL]
                for h in range(NH):
                    q_ps = setps.tile([128, BL], F32, tag="qps")
                    for kt in range(kt_f):
                        nc.tensor.matmul(
                            q_ps[:],
                            Wq_sb[:, kt, h * 128 : (h + 1) * 128],
                            frT_sb[:, kt, :],
                            start=(kt == 0),
                            stop=(kt == kt_f - 1),
                        )
                    nc.vector.tensor_copy(qT_sb[:, h, :], q_ps[:])

                # kT[a_half] = Wk[:, half].T @ confT -> [128, N]
                for h in range(NH):
                    for c in range(N // NCHUNK):
                        k_ps = setps.tile([128, NCHUNK], F32, tag="kps")
                        for kt in range(kt_f):
                            nc.tensor.matmul(
                                k_ps[:],
                                Wk_sb[:, kt, h * 128 : (h + 1) * 128],
                                confT_sb[:, kt, c * NCHUNK : (c + 1) * NCHUNK],
                                start=(kt == 0),
                                stop=(kt == kt_f - 1),
                            )
                        nc.vector.tensor_copy(
                            kT_sb[:, h, c * NCHUNK : (c + 1) * NCHUNK], k_ps[:]
                        )

                # probs broadcast across the 64 batch partitions
                for c in range(N // NCHUNK):
                    pb_ps = setps.tile([BL, NCHUNK], F32, tag="pbps")
                    nc.tensor.matmul(
                        pb_ps[:],
                        ones_sb[:],
                        probs_sb[:, c * NCHUNK : (c + 1) * NCHUNK],
                        start=True,
                        stop=True,
                    )
                    nc.vector.tensor_copy(
                        probs_bc[:, c * NCHUNK : (c + 1) * NCHUNK], pb_ps[:]
                    )

                # onehot[p, h, b, m] = wtT[p, h] * (b == m)
                nc.gpsimd.memset(onehot[:], 0.0)
                nc.gpsimd.affine_select(
                    out=onehot[:],
                    in_=onehot[:],
                    compare_op=mybir.AluOpType.not_equal,
                    fill=1.0,
                    base=0,
                    pattern=[[0, NH], [1, BL], [-1, BL]],
                    channel_multiplier=0,
                )
                for h in range(NH):
                    nc.vector.tensor_scalar_mul(
                        onehot[:, h, :, :], onehot[:, h, :, :], wtT_sb[:, h : h + 1]
                    )

            # ---------------- main loop ----------------
            with tc.tile_pool(name="fusep", bufs=2) as fp:
                for h in range(NH):
                    for gi in range(ngroups):
                        buf = fp.tile([128, g, N], F32, tag="fuse")
                        for j in range(g):
                            bb = gi * g + j
                            nc.vector.tensor_scalar_add(
                                buf[:, j, :],
                                kT_sb[:, h, :],
                                qT_sb[:, h, bb : bb + 1],
                            )
                        nc.scalar.activation(buf[:], buf[:], AF.Tanh)
                        for j in range(g):
                            bb = gi * g + j
                            for c in range(N // NCHUNK):
                                nc.tensor.matmul(
                                    scores_ps[c][:],
                                    onehot[:, h, bb, :],
                                    buf[:, j, c * NCHUNK : (c + 1) * NCHUNK],
                                    start=(h == 0 and bb == 0),
                                    stop=(h == NH - 1 and bb == BL - 1),
                                )

            # ---------------- softmax + weighted sum ----------------
            with (
                tc.tile_pool(name="fin", bufs=1) as fpool,
                tc.tile_pool(name="finps", bufs=2, space="PSUM") as finps,
            ):
                scores_sb = fpool.tile([BL, N], F32)
                for c in range(N // NCHUNK):
                    nc.vector.tensor_copy(
                        scores_sb[:, c * NCHUNK : (c + 1) * NCHUNK], scores_ps[c][:]
                    )
                negmx = fpool.tile([BL, 1], F32)
                nc.vector.tensor_reduce(
                    negmx[:],
                    scores_sb[:],
                    mybir.AxisListType.X,
                    mybir.AluOpType.max,
                    negate=True,
                )
                wexp = fpool.tile([BL, N], F32)
                sums = fpool.tile([BL, 1], F32)
                nc.scalar.activation(
                    wexp[:], scores_sb[:], AF.Exp, bias=negmx[:], accum_out=sums[:]
                )
                recip = fpool.tile([BL, 1], F32)
                nc.vector.reciprocal(recip[:], sums[:])
                w_un = fpool.tile([BL, N], F32)
                nc.vector.tensor_mul(w_un[:], wexp[:], probs_bc[:])

                identity64 = fpool.tile([BL, BL], F32)
                make_identity(nc, identity64[:])
                wT = fpool.tile([128, nt, BL], F32)
                for t in range(nt):
                    tr_ps = finps.tile([128, BL], F32, tag="trps")
                    nc.tensor.transpose(
                        tr_ps[:], w_un[:, t * 128 : (t + 1) * 128], identity64[:]
                    )
                    nc.vector.tensor_copy(wT[:, t, :], tr_ps[:])

                out_sb = fpool.tile([BL, D], F32)
                for dc in range(D // NCHUNK):
                    f_ps = finps.tile([BL, NCHUNK], F32, tag="fps")
                    for t in range(nt):
                        nc.tensor.matmul(
                            f_ps[:],
                            wT[:, t, :],
                            conf_sb[:, t, dc * NCHUNK : (dc + 1) * NCHUNK],
                            start=(t == 0),
                            stop=(t == nt - 1),
                        )
                    nc.scalar.activation(
                        out_sb[:, dc * NCHUNK : (dc + 1) * NCHUNK],
                        f_ps[:],
                        AF.Copy,
                        scale=recip[:],
                    )
                nc.sync.dma_start(out[:], out_sb[:])

    nc.compile()
    return nc


_NC_CACHE = {}


def _get_nc(g: int = G):
    if g not in _NC_CACHE:
        _NC_CACHE[g] = build_kernel(g)
    return _NC_CACHE[g]


def _make_in_maps(inputs):
    conf = np.ascontiguousarray(np.asarray(inputs["confounder_set"], np.float32))
    fr = np.asarray(inputs["fuse_rep"], np.float32)
    probs = np.ascontiguousarray(
        np.asarray(inputs["probabilities"], np.float32).reshape(1, N)
    )
    Wq = np.ascontiguousarray(np.asarray(inputs["Wq"], np.float32))
    Wk = np.ascontiguousarray(np.asarray(inputs["Wk"], np.float32))
    wt = np.asarray(inputs["wt"], np.float32)

    confT = np.ascontiguousarray(conf.T)
    frT = np.ascontiguousarray(fr.T)  # [FUSE, B]
    wtT = np.ascontiguousarray(wt.reshape(NH, 128).T)  # [128, NH]

    in_maps = []
    for c in range(M):
        in_maps.append(
            {
                "conf": conf,
                "confT": confT,
                "frT": np.ascontiguousarray(frT[:, c * BL : (c + 1) * BL]),
                "Wq": Wq,
                "Wk": Wk,
                "probs": probs,
                "wtT": wtT,
            }
        )
    return in_maps


def _run(inputs, trace: bool = False, g: int = G):
    nc = _get_nc(g)
    in_maps = _make_in_maps(inputs)
    res = run_bass_kernel_spmd(nc, in_maps, core_ids=list(range(M)), trace=trace)
    out_full = np.concatenate(
        [res.results[i]["out"] for i in range(M)], axis=0
    ).astype(np.float32)
    return out_full, res


def kernel(**inputs) -> np.ndarray:
    out, _ = _run(inputs)
    return out
